# revision 1
# baseline (speedup 1.0000x reference)
"""ComplEx edge-scoring kernel for Trainium2 (8 NeuronCores, SPMD).

score[e] = Re(<h, r, conj(t)>) for 400k edges gathered from node/rel
embedding tables.

Gather strategy (the crux): the only fast gather primitive on this stack is
the ANT `dma_gather` ucode, whose indices are int16 (< 32768). Node ids go to
100k, so edges are bucketed host-side by (src%4, dst%4) into 16 classes; a
class-(a,b) bucket gathers src rows from a strided view of the node table
(base = a rows, stride = 4 rows) with idx16 = src>>2, and dst rows likewise.
Rel ids (<1000) fit int16 directly.

Host-side, all 400k edges are bucketed globally and dealt round-robin to the
8 cores, so every core gets an identical compile-time layout: 16 buckets
padded to BUCKET_CAP edges each. Scores are un-permuted host-side.

Device layout: gathered edge ordinal n (within a chunk) lands on partition
n%128, block n//128 (dma_gather's native layout). Each chunk = CHUNK edges =
BLOCKS blocks of 128. Per chunk and role one (occasionally two, at bucket
boundaries) dma_gather ops fetch [128, BLOCKS, 256] f32 of embedding rows.

Compute per chunk, with r/i = first/second 128 features:
    pt = [rr*rt | ir*it]          (full-width mult)
    qt = [rr*it | ir*rt]          (two half-width mults)
    ct = [pt.r + pt.i | qt.r - qt.i]
    score_block = sum(h_block * ct_block)   (fused tensor_tensor_reduce)
"""

import sys

if "/opt/trn_rl_repo" not in sys.path:
    sys.path.insert(0, "/opt/trn_rl_repo")

from contextlib import ExitStack

import numpy as np

import concourse.bass as bass
import concourse.tile as tile
from concourse import bacc, mybir
from concourse.bass_utils import run_bass_kernel_spmd

N_NODES = 100000
N_RELS = 1000
N_EDGES = 400000
D = 256
HALF = D // 2
P = 128
NCORES = 8

BUCKET_CAP = 3456        # per-core per-bucket slots (multiple of 128)
NBUCKETS = 16
TOTAL = NBUCKETS * BUCKET_CAP   # 55296 padded edges per core
CHUNK = 1024
BLOCKS = CHUNK // P      # 8
NCH = TOTAL // CHUNK     # 54
NCOLS = TOTAL // P       # 432

F32 = mybir.dt.float32
I16 = mybir.dt.int16

mult = mybir.AluOpType.mult
add = mybir.AluOpType.add
sub = mybir.AluOpType.subtract


def emit_kernel(ctx, tc, table_hdl, idx_ap, scores_ap, *, n_nodes, n_rels,
                cap, chunk, nch, gather_bufs=2, work_bufs=2):
    nc = tc.nc
    blocks = chunk // P
    total = 16 * cap
    ncols = total // P
    qrows = n_nodes // 4

    ipool = ctx.enter_context(tc.tile_pool(name="idxp", bufs=1))
    gpool = ctx.enter_context(tc.tile_pool(name="gath", bufs=gather_bufs))
    wpool = ctx.enter_context(tc.tile_pool(name="work", bufs=work_bufs))
    spool = ctx.enter_context(tc.tile_pool(name="scor", bufs=1))

    idx_cols = total // 16
    idx_sb = ipool.tile([P, 3 * idx_cols], I16)
    nc.sync.dma_start(idx_sb[:], idx_ap)

    s_all = spool.tile([P, ncols], F32)

    def class_of(ordinal, role):
        if role == 0:
            return (ordinal // (4 * cap)) % 4
        if role == 1:
            return (ordinal // cap) % 4
        return None

    def splits(base, role):
        period = 4 * cap if role == 0 else (cap if role == 1 else total)
        out, s = [], base
        while s < base + chunk:
            nxt = min(base + chunk, (s // period + 1) * period)
            out.append((s, nxt - s, class_of(s, role)))
            s = nxt
        return out

    def gather_chunk(c):
        tiles = []
        for role in range(3):
            g = gpool.tile([P, blocks * D], F32, tag=f"g{role}")
            gv = g[:].rearrange("p (b d) -> p b d", d=D)
            for (start, n, cls) in splits(c * chunk, role):
                b0 = (start - c * chunk) // P
                nb = n // P
                if role == 2:
                    in_ap = bass.AP(table_hdl, n_nodes * D,
                                    [[D, n_rels], [1, D]])
                    estep = None
                else:
                    in_ap = bass.AP(table_hdl, cls * D,
                                    [[4 * D, qrows], [1, D]])
                    estep = 4 * D
                nc.gpsimd.dma_gather(
                    out_ap=gv[:, b0:b0 + nb, :],
                    in_ap=in_ap,
                    idxs_ap=idx_sb[:, role * idx_cols + start // 16:
                                   role * idx_cols + (start + n) // 16],
                    num_idxs=n,
                    num_idxs_reg=n,
                    elem_size=D,
                    elem_step=estep,
                    single_packet=False,
                )
            tiles.append(g)
        return tiles

    def halves(ap, blks):
        v = ap.rearrange("p (b two d) -> p b two d", two=2, d=HALF)
        return v[:, :, 0, :], v[:, :, 1, :]

    def compute_chunk(c, tiles):
        gh, gt, gr = (t[:] for t in tiles)
        kd = blocks * D
        t_r, t_i = halves(gt, blocks)
        r_r, r_i = halves(gr, blocks)
        pt = wpool.tile([P, kd], F32, tag="pt")
        nc.vector.tensor_tensor(out=pt[:], in0=gt, in1=gr, op=mult)
        qt = wpool.tile([P, kd], F32, tag="qt")
        q_r, q_i = halves(qt[:], blocks)
        nc.vector.tensor_tensor(out=q_r, in0=r_r, in1=t_i, op=mult)
        nc.vector.tensor_tensor(out=q_i, in0=r_i, in1=t_r, op=mult)
        ct = wpool.tile([P, kd], F32, tag="ct")
        c_r, c_i = halves(ct[:], blocks)
        p_r, p_i = halves(pt[:], blocks)
        nc.vector.tensor_tensor(out=c_r, in0=p_r, in1=p_i, op=add)
        nc.vector.tensor_tensor(out=c_i, in0=q_r, in1=q_i, op=sub)
        scratch = wpool.tile([P, kd], F32, tag="sc")
        for b in range(blocks):
            nc.vector.scalar_tensor_tensor(
                out=scratch[:, b * D:(b + 1) * D],
                in0=gh[:, b * D:(b + 1) * D],
                scalar=1.0,
                in1=ct[:, b * D:(b + 1) * D],
                op0=mult,
                op1=mult,
                accum_out=s_all[:, c * blocks + b:c * blocks + b + 1],
            )

    pending = gather_chunk(0)
    for c in range(nch):
        cur = pending
        if c + 1 < nch:
            pending = gather_chunk(c + 1)
        compute_chunk(c, cur)

    nc.sync.dma_start(scores_ap, s_all[:])


def build_program(*, n_nodes=N_NODES, n_rels=N_RELS, cap=BUCKET_CAP,
                  chunk=CHUNK, num_devices=NCORES, gather_bufs=2,
                  work_bufs=2, enable_asserts=False, num_swdge_queues=1):
    total = 16 * cap
    nch = total // chunk
    nc = bacc.Bacc(
        "TRN2",
        target_bir_lowering=False,
        debug=False,
        enable_asserts=enable_asserts,
        num_devices=num_devices,
        num_swdge_queues=num_swdge_queues,
    )
    table_h = nc.dram_tensor("table", [n_nodes + n_rels, D], F32,
                             kind="ExternalInput")
    idx = nc.dram_tensor("idx16", [P, 3 * (total // 16)], I16,
                         kind="ExternalInput").ap()
    scores = nc.dram_tensor("scores", [P, total // P], F32,
                            kind="ExternalOutput").ap()
    with tile.TileContext(nc) as tc, ExitStack() as ctx:
        emit_kernel(ctx, tc, table_h, idx, scores, n_nodes=n_nodes,
                    n_rels=n_rels, cap=cap, chunk=chunk, nch=nch,
                    gather_bufs=gather_bufs, work_bufs=work_bufs)
    nc.compile()
    return nc


def _wrap16(vals):
    """[total] int16 -> [128, total/16] wrapped layout (ordinal n at
    partition n%16, col n//16; replicated across the 8 Q7 core groups)."""
    w = vals.reshape(-1, 16).T.astype(np.int16)
    return np.tile(w, (8, 1))


def pack_inputs(node_emb, rel_emb, src, dst, rel_id, *, cap=BUCKET_CAP):
    """Bucket + deal edges to cores; build per-core idx16 arrays.

    Returns (table, per_core_idx16 list, slot_edge list).
    """
    total = 16 * cap
    key = (src % 4) * 4 + (dst % 4)
    order = np.argsort(key, kind="stable")
    sorted_key = key[order]
    bucket_starts = np.searchsorted(sorted_key, np.arange(17))
    table = np.concatenate([node_emb, rel_emb], axis=0)

    per_core_slots = []
    for core in range(NCORES):
        slots = np.empty(total, dtype=np.int64)
        for b in range(16):
            members = order[bucket_starts[b] + core:bucket_starts[b + 1]:NCORES]
            assert 0 < len(members) <= cap, (
                f"bucket {b} core {core}: {len(members)} > cap {cap}"
            )
            padded = np.full(cap, members[-1], dtype=np.int64)
            padded[:len(members)] = members
            slots[b * cap:(b + 1) * cap] = padded
        per_core_slots.append(slots)

    per_core_idx = []
    for slots in per_core_slots:
        s, d_, r = src[slots], dst[slots], rel_id[slots]
        idx16 = np.concatenate([
            _wrap16(s >> 2), _wrap16(d_ >> 2), _wrap16(r)], axis=1)
        per_core_idx.append(np.ascontiguousarray(idx16))
    return table, per_core_idx, per_core_slots


_PROGRAM_CACHE = {}

# test-harness hooks: test.py sets _RUN_KWARGS["trace"]=True and reads _LAST
_RUN_KWARGS = {}
_LAST = {}


def _get_program():
    if "nc" not in _PROGRAM_CACHE:
        _PROGRAM_CACHE["nc"] = build_program()
    return _PROGRAM_CACHE["nc"]


def kernel(node_emb, rel_emb, src, dst, rel_id):
    node_emb = np.ascontiguousarray(np.asarray(node_emb, dtype=np.float32))
    rel_emb = np.ascontiguousarray(np.asarray(rel_emb, dtype=np.float32))
    src = np.asarray(src).astype(np.int64)
    dst = np.asarray(dst).astype(np.int64)
    rel_id = np.asarray(rel_id).astype(np.int64)

    table, per_core_idx, per_core_slots = pack_inputs(
        node_emb, rel_emb, src, dst, rel_id)
    nc = _get_program()

    in_maps = [
        {"table": table, "idx16": per_core_idx[m]} for m in range(NCORES)
    ]
    res = run_bass_kernel_spmd(nc, in_maps, core_ids=list(range(NCORES)),
                               **_RUN_KWARGS)
    _LAST["res"] = res

    out = np.empty(N_EDGES, dtype=np.float32)
    for m in range(NCORES):
        scores_sorted = res.results[m]["scores"].T.reshape(-1)  # [TOTAL]
        out[per_core_slots[m]] = scores_sorted
    return out



# revision 2
# speedup vs baseline: 1.7732x; 1.7732x over previous
"""ComplEx edge-scoring kernel for Trainium2 (8 NeuronCores, SPMD) — v2.

score[e] = Re(<h, r, conj(t)>) for 400k edges.

Bottleneck analysis (v1): all three roles (src/dst/rel) were fetched with the
gpsimd dma_gather ucode, whose descriptor generation runs on ONE Q7 cpu pair
at ~9ns/row; 3 x 55k rows/core serialized to ~1.45ms.

v2 changes:
  * Edges are routed to cores by rel ownership (core m owns rels
    [125m, 125(m+1))), so each core sees only 125 distinct rel rows. The rel
    "gather" becomes a one-hot matmul on the (idle) tensor engine:
    r_block = OH_block^T @ W_own, OH host-precomputed — zero descriptors.
  * Tables in bf16: halves gather DMA bytes and vector-engine cost.
  * Only src/dst go through dma_gather (2 streams instead of 3).

Layout: per core, edges are bucketed by (src%4, dst%4) into 16 classes so
idx16 fits int16 (idx = id>>2, table viewed with stride 4 rows, base = class).
Slot ordinal n -> partition n%128, block n//128. Scores unpermuted host-side.
"""

import sys

if "/opt/trn_rl_repo" not in sys.path:
    sys.path.insert(0, "/opt/trn_rl_repo")

from contextlib import ExitStack

import ml_dtypes
import numpy as np

import concourse.bass as bass
import concourse.tile as tile
from concourse import bacc, mybir
from concourse.bass_utils import run_bass_kernel_spmd

N_NODES = 100000
N_RELS = 1000
N_EDGES = 400000
D = 256
HALF = D // 2
P = 128
NCORES = 8
RPC = N_RELS // NCORES  # 125 rels per core

CHUNK = 1024
BLOCKS = CHUNK // P  # 8

F32 = mybir.dt.float32
BF16 = mybir.dt.bfloat16
I16 = mybir.dt.int16

mult = mybir.AluOpType.mult
add = mybir.AluOpType.add
sub = mybir.AluOpType.subtract

BF = ml_dtypes.bfloat16


def emit_kernel(ctx, tc, table_h, wrel_ap, idx_ap, oh_ap, scores_ap, *, cap,
                chunk, nch):
    nc = tc.nc
    blocks = chunk // P
    total = 16 * cap
    ncols = total // P
    qrows = N_NODES // 4
    idx_cols = total // 16

    ipool = ctx.enter_context(tc.tile_pool(name="idxp", bufs=1))
    cpool = ctx.enter_context(tc.tile_pool(name="const", bufs=1))
    opool = ctx.enter_context(tc.tile_pool(name="ohp", bufs=2))
    gpool = ctx.enter_context(tc.tile_pool(name="gath", bufs=2))
    ppool = ctx.enter_context(tc.tile_pool(name="psum", bufs=2, space="PSUM"))
    wpool = ctx.enter_context(tc.tile_pool(name="work", bufs=2))
    spool = ctx.enter_context(tc.tile_pool(name="scor", bufs=1))

    idx_sb = ipool.tile([P, 2 * idx_cols], I16)
    nc.sync.dma_start(idx_sb[:], idx_ap)
    wrel_sb = cpool.tile([P, D], BF16)
    nc.sync.dma_start(wrel_sb[:], wrel_ap)

    s_all = spool.tile([P, ncols], F32)

    def class_of(ordinal, role):
        if role == 0:
            return (ordinal // (4 * cap)) % 4
        return (ordinal // cap) % 4

    def splits(base, role):
        # split at bucket boundaries (period=cap) for BOTH roles so padded
        # slots (idx16 = -1) are always trailing within a call and get
        # runtime-trimmed by the ucode
        out, s = [], base
        while s < base + chunk:
            nxt = min(base + chunk, (s // cap + 1) * cap)
            out.append((s, nxt - s, class_of(s, role)))
            s = nxt
        return out

    def gather_chunk(c):
        tiles = []
        for role in range(2):
            g = gpool.tile([P, blocks * D], BF16, tag=f"g{role}")
            gv = g[:].rearrange("p (b d) -> p b d", d=D)
            for (start, n, cls) in splits(c * chunk, role):
                b0 = (start - c * chunk) // P
                nb = n // P
                in_ap = bass.AP(table_h, cls * D, [[4 * D, qrows], [1, D]])
                nc.gpsimd.dma_gather(
                    out_ap=gv[:, b0:b0 + nb, :],
                    in_ap=in_ap,
                    idxs_ap=idx_sb[:, role * idx_cols + start // 16:
                                   role * idx_cols + (start + n) // 16],
                    num_idxs=n,
                    num_idxs_reg=n,
                    elem_size=D,
                    elem_step=4 * D,
                    single_packet=False,
                )
            tiles.append(g)
        return tiles

    def rel_chunk(c, oh_sb):
        r_ps = ppool.tile([P, blocks * D], F32, tag="rps")
        rv = r_ps[:].rearrange("p (b d) -> p b d", d=D)
        for b in range(blocks):
            nc.tensor.matmul(
                out=rv[:, b, :],
                lhsT=oh_sb[:RPC, b * P:(b + 1) * P],
                rhs=wrel_sb[:RPC, :],
                start=True,
                stop=True,
            )
        gr = wpool.tile([P, blocks * D], BF16, tag="gr")
        nc.vector.tensor_copy(out=gr[:], in_=r_ps[:])
        return gr

    def halves(ap):
        v = ap.rearrange("p (b two d) -> p b two d", two=2, d=HALF)
        return v[:, :, 0, :], v[:, :, 1, :]

    def compute_chunk(c, gh_t, gt_t, gr_t):
        gh, gt, gr = gh_t[:], gt_t[:], gr_t[:]
        kd = blocks * D
        t_r, t_i = halves(gt)
        r_r, r_i = halves(gr)
        pt = wpool.tile([P, kd], BF16, tag="pt")
        nc.vector.tensor_tensor(out=pt[:], in0=gt, in1=gr, op=mult)
        qt = wpool.tile([P, kd], BF16, tag="qt")
        q_r, q_i = halves(qt[:])
        nc.vector.tensor_tensor(out=q_r, in0=r_r, in1=t_i, op=mult)
        nc.vector.tensor_tensor(out=q_i, in0=r_i, in1=t_r, op=mult)
        ct = wpool.tile([P, kd], BF16, tag="ct")
        c_r, c_i = halves(ct[:])
        p_r, p_i = halves(pt[:])
        nc.vector.tensor_tensor(out=c_r, in0=p_r, in1=p_i, op=add)
        nc.vector.tensor_tensor(out=c_i, in0=q_r, in1=q_i, op=sub)
        scratch = wpool.tile([P, kd], BF16, tag="sc")
        for b in range(blocks):
            nc.vector.scalar_tensor_tensor(
                out=scratch[:, b * D:(b + 1) * D],
                in0=gh[:, b * D:(b + 1) * D],
                scalar=1.0,
                in1=ct[:, b * D:(b + 1) * D],
                op0=mult,
                op1=mult,
                accum_out=s_all[:, c * blocks + b:c * blocks + b + 1],
            )

    # software pipeline: prefetch gathers + oh of chunk c+1 during compute c
    def load_oh(c):
        oh_sb = opool.tile([P, chunk], BF16, tag="oh")
        nc.sync.dma_start(oh_sb[:], oh_ap[:, c * chunk:(c + 1) * chunk])
        return oh_sb

    pend_g = gather_chunk(0)
    pend_oh = load_oh(0)
    for c in range(nch):
        gh_t, gt_t = pend_g
        oh_sb = pend_oh
        if c + 1 < nch:
            pend_g = gather_chunk(c + 1)
            pend_oh = load_oh(c + 1)
        gr_t = rel_chunk(c, oh_sb)
        compute_chunk(c, gh_t, gt_t, gr_t)

    nc.sync.dma_start(scores_ap, s_all[:])


def build_program(*, cap, num_devices=NCORES, num_swdge_queues=1):
    total = 16 * cap
    nch = total // CHUNK
    nc = bacc.Bacc(
        "TRN2",
        target_bir_lowering=False,
        debug=False,
        enable_asserts=False,
        num_devices=num_devices,
        num_swdge_queues=num_swdge_queues,
    )
    table_h = nc.dram_tensor("table", [N_NODES, D], BF16, kind="ExternalInput")
    wrel = nc.dram_tensor("wrel", [P, D], BF16, kind="ExternalInput").ap()
    idx = nc.dram_tensor("idx16", [P, 2 * (total // 16)], I16,
                         kind="ExternalInput").ap()
    oh = nc.dram_tensor("oh", [P, total], BF16, kind="ExternalInput").ap()
    scores = nc.dram_tensor("scores", [P, total // P], F32,
                            kind="ExternalOutput").ap()
    with tile.TileContext(nc) as tc, ExitStack() as ctx:
        emit_kernel(ctx, tc, table_h, wrel, idx, oh, scores, cap=cap,
                    chunk=CHUNK, nch=nch)
    nc.compile()
    return nc


def _wrap16(vals):
    w = vals.reshape(-1, 16).T.astype(np.int16)
    return np.tile(w, (8, 1))


def pack_inputs(node_emb, rel_emb, src, dst, rel_id):
    """Route edges by rel owner; bucket by (src%4,dst%4); build per-core arrays.

    Returns (table_bf, per_core dicts, per_core_slots, cap).
    """
    owner = rel_id // RPC
    key = (src % 4) * 4 + (dst % 4)

    per_core_edges = [np.where(owner == m)[0] for m in range(NCORES)]

    # global cap: max bucket size over all cores, rounded up to 64
    needed = 0
    per_core_buckets = []
    for m in range(NCORES):
        e = per_core_edges[m]
        k = key[e]
        order = np.argsort(k, kind="stable")
        ke = k[order]
        starts = np.searchsorted(ke, np.arange(17))
        sizes = np.diff(starts)
        needed = max(needed, int(sizes.max()))
        per_core_buckets.append((e[order], starts))
    cap = ((needed + 127) // 128) * 128
    total = 16 * cap

    table_bf = np.ascontiguousarray(node_emb.astype(BF))
    rel_bf = rel_emb.astype(BF)

    cores = []
    per_core_slots = []
    per_core_valid = []
    for m in range(NCORES):
        eo, starts = per_core_buckets[m]
        slots = np.empty(total, dtype=np.int64)
        valid = np.zeros(total, dtype=bool)
        for b in range(16):
            members = eo[starts[b]:starts[b + 1]]
            fallback = members[-1] if len(members) else (eo[0] if len(eo) else 0)
            padded = np.full(cap, fallback, dtype=np.int64)
            padded[:len(members)] = members
            slots[b * cap:(b + 1) * cap] = padded
            valid[b * cap:b * cap + len(members)] = True
        per_core_slots.append(slots)
        per_core_valid.append(valid)

        idx16 = np.concatenate([_wrap16(src[slots] >> 2),
                                _wrap16(dst[slots] >> 2)], axis=1)

        lid = np.where(valid, rel_id[slots] - m * RPC, 0).astype(np.int64)
        assert lid.min() >= 0 and lid.max() < RPC, (lid.min(), lid.max())
        oh = (np.arange(P)[:, None] == lid[None, :]).astype(BF)

        wrel = np.zeros((P, D), dtype=BF)
        wrel[:RPC] = rel_bf[m * RPC:(m + 1) * RPC]

        cores.append({
            "table": table_bf,
            "wrel": np.ascontiguousarray(wrel),
            "idx16": np.ascontiguousarray(idx16),
            "oh": np.ascontiguousarray(oh),
        })
    return cores, per_core_slots, per_core_valid, cap


_PROGRAM_CACHE = {}

_RUN_KWARGS = {}
_LAST = {}


def _get_program(cap):
    if cap not in _PROGRAM_CACHE:
        _PROGRAM_CACHE[cap] = build_program(cap=cap)
    return _PROGRAM_CACHE[cap]


def kernel(node_emb, rel_emb, src, dst, rel_id):
    node_emb = np.asarray(node_emb, dtype=np.float32)
    rel_emb = np.asarray(rel_emb, dtype=np.float32)
    src = np.asarray(src).astype(np.int64)
    dst = np.asarray(dst).astype(np.int64)
    rel_id = np.asarray(rel_id).astype(np.int64)

    cores, per_core_slots, per_core_valid, cap = pack_inputs(
        node_emb, rel_emb, src, dst, rel_id)
    nc = _get_program(cap)

    res = run_bass_kernel_spmd(nc, cores, core_ids=list(range(NCORES)),
                               **_RUN_KWARGS)
    _LAST["res"] = res

    out = np.empty(N_EDGES, dtype=np.float32)
    for m in range(NCORES):
        scores_sorted = res.results[m]["scores"].T.reshape(-1)
        v = per_core_valid[m]
        out[per_core_slots[m][v]] = scores_sorted[v]
    return out


# revision 3
# speedup vs baseline: 1.8070x; 1.0190x over previous
"""ComplEx edge-scoring kernel for Trainium2 (8 NeuronCores, SPMD) — v2.

score[e] = Re(<h, r, conj(t)>) for 400k edges.

Bottleneck analysis (v1): all three roles (src/dst/rel) were fetched with the
gpsimd dma_gather ucode, whose descriptor generation runs on ONE Q7 cpu pair
at ~9ns/row; 3 x 55k rows/core serialized to ~1.45ms.

v2 changes:
  * Edges are routed to cores by rel ownership (core m owns rels
    [125m, 125(m+1))), so each core sees only 125 distinct rel rows. The rel
    "gather" becomes a one-hot matmul on the (idle) tensor engine:
    r_block = OH_block^T @ W_own, OH host-precomputed — zero descriptors.
  * Tables in bf16: halves gather DMA bytes and vector-engine cost.
  * Only src/dst go through dma_gather (2 streams instead of 3).

Layout: per core, edges are bucketed by (src%4, dst%4) into 16 classes so
idx16 fits int16 (idx = id>>2, table viewed with stride 4 rows, base = class).
Slot ordinal n -> partition n%128, block n//128. Scores unpermuted host-side.
"""

import sys

if "/opt/trn_rl_repo" not in sys.path:
    sys.path.insert(0, "/opt/trn_rl_repo")

from contextlib import ExitStack

import ml_dtypes
import numpy as np

import concourse.bass as bass
import concourse.tile as tile
from concourse import bacc, mybir
from concourse.bass_utils import run_bass_kernel_spmd

N_NODES = 100000
N_RELS = 1000
N_EDGES = 400000
D = 256
HALF = D // 2
P = 128
NCORES = 8
RPC = N_RELS // NCORES  # 125 rels per core

CHUNK = 1024
BLOCKS = CHUNK // P  # 8

F32 = mybir.dt.float32
BF16 = mybir.dt.bfloat16
I16 = mybir.dt.int16

mult = mybir.AluOpType.mult
add = mybir.AluOpType.add
sub = mybir.AluOpType.subtract

BF = ml_dtypes.bfloat16


def emit_kernel(ctx, tc, table_h, wrel_ap, idx_ap, oh_ap, scores_ap, *, cap,
                chunk, nch):
    nc = tc.nc
    blocks = chunk // P
    total = 16 * cap
    ncols = total // P
    qrows = N_NODES // 4
    idx_cols = total // 16

    ipool = ctx.enter_context(tc.tile_pool(name="idxp", bufs=1))
    cpool = ctx.enter_context(tc.tile_pool(name="const", bufs=1))
    opool = ctx.enter_context(tc.tile_pool(name="ohp", bufs=2))
    gpool = ctx.enter_context(tc.tile_pool(name="gath", bufs=2))
    ppool = ctx.enter_context(tc.tile_pool(name="psum", bufs=2, space="PSUM"))
    wpool = ctx.enter_context(tc.tile_pool(name="work", bufs=2))
    spool = ctx.enter_context(tc.tile_pool(name="scor", bufs=1))

    idx_sb = ipool.tile([P, 2 * idx_cols], I16)
    nc.sync.dma_start(idx_sb[:], idx_ap)
    wrel_sb = cpool.tile([P, D], BF16)
    nc.sync.dma_start(wrel_sb[:], wrel_ap)

    s_all = spool.tile([P, ncols], F32)

    def class_of(ordinal, role):
        if role == 0:
            return (ordinal // (4 * cap)) % 4
        return (ordinal // cap) % 4

    def splits(base, role):
        # split at bucket boundaries (period=cap) for BOTH roles so padded
        # slots (idx16 = -1) are always trailing within a call and get
        # runtime-trimmed by the ucode
        out, s = [], base
        while s < base + chunk:
            nxt = min(base + chunk, (s // cap + 1) * cap)
            out.append((s, nxt - s, class_of(s, role)))
            s = nxt
        return out

    def gather_chunk(c):
        tiles = []
        for role in range(2):
            g = gpool.tile([P, blocks * D], BF16, tag=f"g{role}")
            gv = g[:].rearrange("p (b d) -> p b d", d=D)
            for (start, n, cls) in splits(c * chunk, role):
                b0 = (start - c * chunk) // P
                nb = n // P
                in_ap = bass.AP(table_h, cls * D, [[4 * D, qrows], [1, D]])
                nc.gpsimd.dma_gather(
                    out_ap=gv[:, b0:b0 + nb, :],
                    in_ap=in_ap,
                    idxs_ap=idx_sb[:, role * idx_cols + start // 16:
                                   role * idx_cols + (start + n) // 16],
                    num_idxs=n,
                    num_idxs_reg=n,
                    elem_size=D,
                    elem_step=4 * D,
                    single_packet=False,
                    queue_num=role,
                )
            tiles.append(g)
        return tiles

    def rel_chunk(c, oh_sb):
        r_ps = ppool.tile([P, blocks * D], F32, tag="rps")
        rv = r_ps[:].rearrange("p (b d) -> p b d", d=D)
        for b in range(blocks):
            nc.tensor.matmul(
                out=rv[:, b, :],
                lhsT=oh_sb[:RPC, b * P:(b + 1) * P],
                rhs=wrel_sb[:RPC, :],
                start=True,
                stop=True,
            )
        gr = wpool.tile([P, blocks * D], BF16, tag="gr")
        nc.vector.tensor_copy(out=gr[:], in_=r_ps[:])
        return gr

    def halves(ap):
        v = ap.rearrange("p (b two d) -> p b two d", two=2, d=HALF)
        return v[:, :, 0, :], v[:, :, 1, :]

    def compute_chunk(c, gh_t, gt_t, gr_t):
        gh, gt, gr = gh_t[:], gt_t[:], gr_t[:]
        kd = blocks * D
        t_r, t_i = halves(gt)
        r_r, r_i = halves(gr)
        pt = wpool.tile([P, kd], BF16, tag="pt")
        nc.vector.tensor_tensor(out=pt[:], in0=gt, in1=gr, op=mult)
        qt = wpool.tile([P, kd], BF16, tag="qt")
        q_r, q_i = halves(qt[:])
        nc.vector.tensor_tensor(out=q_r, in0=r_r, in1=t_i, op=mult)
        nc.vector.tensor_tensor(out=q_i, in0=r_i, in1=t_r, op=mult)
        ct = wpool.tile([P, kd], BF16, tag="ct")
        c_r, c_i = halves(ct[:])
        p_r, p_i = halves(pt[:])
        nc.vector.tensor_tensor(out=c_r, in0=p_r, in1=p_i, op=add)
        nc.vector.tensor_tensor(out=c_i, in0=q_r, in1=q_i, op=sub)
        scratch = wpool.tile([P, kd], BF16, tag="sc")
        for b in range(blocks):
            nc.vector.scalar_tensor_tensor(
                out=scratch[:, b * D:(b + 1) * D],
                in0=gh[:, b * D:(b + 1) * D],
                scalar=1.0,
                in1=ct[:, b * D:(b + 1) * D],
                op0=mult,
                op1=mult,
                accum_out=s_all[:, c * blocks + b:c * blocks + b + 1],
            )

    # software pipeline: prefetch gathers + oh of chunk c+1 during compute c
    def load_oh(c):
        oh_sb = opool.tile([P, chunk], BF16, tag="oh")
        nc.sync.dma_start(oh_sb[:], oh_ap[:, c * chunk:(c + 1) * chunk])
        return oh_sb

    pend_g = gather_chunk(0)
    pend_oh = load_oh(0)
    for c in range(nch):
        gh_t, gt_t = pend_g
        oh_sb = pend_oh
        if c + 1 < nch:
            pend_g = gather_chunk(c + 1)
            pend_oh = load_oh(c + 1)
        gr_t = rel_chunk(c, oh_sb)
        compute_chunk(c, gh_t, gt_t, gr_t)

    nc.sync.dma_start(scores_ap, s_all[:])


def build_program(*, cap, num_devices=NCORES, num_swdge_queues=1):
    total = 16 * cap
    nch = total // CHUNK
    nc = bacc.Bacc(
        "TRN2",
        target_bir_lowering=False,
        debug=False,
        enable_asserts=False,
        num_devices=num_devices,
        num_swdge_queues=num_swdge_queues,
    )
    table_h = nc.dram_tensor("table", [N_NODES, D], BF16, kind="ExternalInput")
    wrel = nc.dram_tensor("wrel", [P, D], BF16, kind="ExternalInput").ap()
    idx = nc.dram_tensor("idx16", [P, 2 * (total // 16)], I16,
                         kind="ExternalInput").ap()
    oh = nc.dram_tensor("oh", [P, total], BF16, kind="ExternalInput").ap()
    scores = nc.dram_tensor("scores", [P, total // P], F32,
                            kind="ExternalOutput").ap()
    with tile.TileContext(nc) as tc, ExitStack() as ctx:
        emit_kernel(ctx, tc, table_h, wrel, idx, oh, scores, cap=cap,
                    chunk=CHUNK, nch=nch)
    nc.compile()
    return nc


def _wrap16(vals):
    w = vals.reshape(-1, 16).T.astype(np.int16)
    return np.tile(w, (8, 1))


def pack_inputs(node_emb, rel_emb, src, dst, rel_id):
    """Route edges by rel owner; bucket by (src%4,dst%4); build per-core arrays.

    Returns (table_bf, per_core dicts, per_core_slots, cap).
    """
    owner = rel_id // RPC
    key = (src % 4) * 4 + (dst % 4)

    per_core_edges = [np.where(owner == m)[0] for m in range(NCORES)]

    # global cap: max bucket size over all cores, rounded up to 64
    needed = 0
    per_core_buckets = []
    for m in range(NCORES):
        e = per_core_edges[m]
        k = key[e]
        order = np.argsort(k, kind="stable")
        ke = k[order]
        starts = np.searchsorted(ke, np.arange(17))
        sizes = np.diff(starts)
        needed = max(needed, int(sizes.max()))
        per_core_buckets.append((e[order], starts))
    cap = ((needed + 127) // 128) * 128
    total = 16 * cap

    table_bf = np.ascontiguousarray(node_emb.astype(BF))
    rel_bf = rel_emb.astype(BF)

    cores = []
    per_core_slots = []
    per_core_valid = []
    for m in range(NCORES):
        eo, starts = per_core_buckets[m]
        slots = np.empty(total, dtype=np.int64)
        valid = np.zeros(total, dtype=bool)
        for b in range(16):
            members = eo[starts[b]:starts[b + 1]]
            fallback = members[-1] if len(members) else (eo[0] if len(eo) else 0)
            padded = np.full(cap, fallback, dtype=np.int64)
            padded[:len(members)] = members
            slots[b * cap:(b + 1) * cap] = padded
            valid[b * cap:b * cap + len(members)] = True
        per_core_slots.append(slots)
        per_core_valid.append(valid)

        idx16 = np.concatenate([_wrap16(src[slots] >> 2),
                                _wrap16(dst[slots] >> 2)], axis=1)

        lid = np.where(valid, rel_id[slots] - m * RPC, 0).astype(np.int64)
        assert lid.min() >= 0 and lid.max() < RPC, (lid.min(), lid.max())
        oh = (np.arange(P)[:, None] == lid[None, :]).astype(BF)

        wrel = np.zeros((P, D), dtype=BF)
        wrel[:RPC] = rel_bf[m * RPC:(m + 1) * RPC]

        cores.append({
            "table": table_bf,
            "wrel": np.ascontiguousarray(wrel),
            "idx16": np.ascontiguousarray(idx16),
            "oh": np.ascontiguousarray(oh),
        })
    return cores, per_core_slots, per_core_valid, cap


_PROGRAM_CACHE = {}

_RUN_KWARGS = {}
_LAST = {}


def _get_program(cap):
    if cap not in _PROGRAM_CACHE:
        _PROGRAM_CACHE[cap] = build_program(cap=cap, num_swdge_queues=2)
    return _PROGRAM_CACHE[cap]


def kernel(node_emb, rel_emb, src, dst, rel_id):
    node_emb = np.asarray(node_emb, dtype=np.float32)
    rel_emb = np.asarray(rel_emb, dtype=np.float32)
    src = np.asarray(src).astype(np.int64)
    dst = np.asarray(dst).astype(np.int64)
    rel_id = np.asarray(rel_id).astype(np.int64)

    cores, per_core_slots, per_core_valid, cap = pack_inputs(
        node_emb, rel_emb, src, dst, rel_id)
    nc = _get_program(cap)

    res = run_bass_kernel_spmd(nc, cores, core_ids=list(range(NCORES)),
                               **_RUN_KWARGS)
    _LAST["res"] = res

    out = np.empty(N_EDGES, dtype=np.float32)
    for m in range(NCORES):
        scores_sorted = res.results[m]["scores"].T.reshape(-1)
        v = per_core_valid[m]
        out[per_core_slots[m][v]] = scores_sorted[v]
    return out
